# revision 1
# baseline (speedup 1.0000x reference)
"""CrossAttention kernel for Trainium2 (8 NeuronCores, SPMD).

Reference math (B=4, C=256, N=4096, OUT=256, TEMP=sqrt(OUT)=16):
    q = Wq @ x          (B, OUT, N)
    k = Wk @ xx         (B, OUT, N)
    v = Wv @ xx         (B, OUT, N)
    attn = softmax(q^T k / TEMP, axis=-1)   (B, N, N)
    y = einsum('bnm,bom->bon', attn, v)     (B, OUT, N)

Sharding: 8 cores = (batch b, query-half h); each core computes its 2048
query rows against the full 4096 keys of its batch.

Per-core kernel (all matmuls bf16 in / fp32 PSUM accumulate):
    q   = (Wq/TEMP)^T-style matmul -> (OUT, bc)      [o on partitions]
    k   = Wk matmul                -> (OUT, m)       [o on partitions]
    vT  = xx^T @ Wv^T (+ones col)  -> (m, OUT+1)     [m on partitions]
    S_T = k-tiles^T @ q            -> (m, bc)        [m on partitions]
    P_T = exp(S_T)    (bf16; logits are O(5) so no max-subtraction needed)
    yT  = P_T-tiles^T @ vT_aug     -> (bc, OUT+1)    [n on partitions]
          last column = softmax denominator (from the ones column)
    y   = transpose(yT[:, :OUT] * 1/yT[:, OUT])      [o on partitions]
"""

import numpy as np
import ml_dtypes
from contextlib import ExitStack

import concourse.bass as bass
import concourse.tile as tile
from concourse import bacc, mybir
from concourse.bass_utils import run_bass_kernel_spmd

B, C, NSEQ, OUT = 4, 256, 4096, 256
TEMP = float(OUT) ** 0.5
NCORES = 8
BF16 = mybir.dt.bfloat16
F32 = mybir.dt.float32
BFNP = ml_dtypes.bfloat16

EXP = mybir.ActivationFunctionType.Exp


def build(bc=2048, m=4096, nblk=512):
    """Build the per-core SPMD Bass program.

    bc: query rows per core; m: key count; nblk: query block width for the
    attention phases (multiple of 128; nblk*4B <= one PSUM bank).
    """
    ct = C // 128     # contraction tiles for the 1x1-conv projections
    ot = OUT // 128   # output-channel tiles
    mt = m // 128     # key tiles
    nb = bc // nblk   # query blocks
    nt = nblk // 128  # 128-query tiles per block
    qch = bc // 512
    kch = m // 512

    nc = bacc.Bacc("TRN2", target_bir_lowering=False, debug=False,
                   num_devices=NCORES)
    xq_d = nc.dram_tensor("xq", [ct, 128, bc], BF16, kind="ExternalInput")
    xkv_d = nc.dram_tensor("xkv", [ct, 128, m], BF16, kind="ExternalInput")
    wq_d = nc.dram_tensor("wqT", [ct, 128, OUT], BF16, kind="ExternalInput")
    wk_d = nc.dram_tensor("wkT", [ct, 128, OUT], BF16, kind="ExternalInput")
    wv_d = nc.dram_tensor("wvT", [ct, 128, OUT], BF16, kind="ExternalInput")
    id_d = nc.dram_tensor("ident", [128, 128], F32, kind="ExternalInput")
    y_d = nc.dram_tensor("y", [ot, 128, bc], F32, kind="ExternalOutput")

    with tile.TileContext(nc) as tc, ExitStack() as ctx:
        const = ctx.enter_context(tc.tile_pool(name="const", bufs=1))

        xq_sb = const.tile([128, ct, bc], BF16, name="xq_sb")
        xkv_sb = const.tile([128, ct, m], BF16, name="xkv_sb")
        wq_sb = const.tile([128, ct, OUT], BF16, name="wq_sb")
        wk_sb = const.tile([128, ct, OUT], BF16, name="wk_sb")
        wv_sb = const.tile([128, ct, OUT], BF16, name="wv_sb")
        ident_sb = const.tile([128, 128], F32, name="ident_sb")
        zbias = const.tile([128, 1], F32, name="zbias")
        q_sb = const.tile([128, ot, bc], BF16, name="q_sb")
        k_sb = const.tile([128, ot, m], BF16, name="k_sb")
        v_sb = const.tile([128, mt, OUT + 1], BF16, name="v_sb")
        y_sb = const.tile([128, ot, bc], F32, name="y_sb")

        for i in range(ct):
            nc.sync.dma_start(xq_sb[:, i, :], xq_d.ap()[i])
            nc.sync.dma_start(xkv_sb[:, i, :], xkv_d.ap()[i])
            nc.sync.dma_start(wq_sb[:, i, :], wq_d.ap()[i])
            nc.sync.dma_start(wk_sb[:, i, :], wk_d.ap()[i])
            nc.sync.dma_start(wv_sb[:, i, :], wv_d.ap()[i])
        nc.sync.dma_start(ident_sb[:], id_d.ap())
        nc.vector.memset(zbias[:], 0.0)
        nc.vector.memset(v_sb[:, :, OUT:OUT + 1], 1.0)

        # ---- q / k / vT projections ----
        with tc.tile_pool(name="qk_ps", bufs=3, space="PSUM") as qk_pool, \
             tc.tile_pool(name="v_ps", bufs=2, space="PSUM") as v_pool:
            for w_sb, x_sb, dst_sb, nch in ((wq_sb, xq_sb, q_sb, qch),
                                            (wk_sb, xkv_sb, k_sb, kch)):
                for o in range(ot):
                    for ch in range(nch):
                        ps = qk_pool.tile([128, 512], F32, tag="qk", name="qk_t")
                        for c in range(ct):
                            nc.tensor.matmul(
                                ps[:], w_sb[:, c, o * 128:(o + 1) * 128],
                                x_sb[:, c, ch * 512:(ch + 1) * 512],
                                start=(c == 0), stop=(c == ct - 1))
                        nc.any.tensor_copy(
                            dst_sb[:, o, ch * 512:(ch + 1) * 512], ps[:])
            for mi in range(mt):
                ps = v_pool.tile([128, OUT], F32, tag="v", name="v_t")
                for c in range(ct):
                    nc.tensor.matmul(
                        ps[:], xkv_sb[:, c, mi * 128:(mi + 1) * 128],
                        wv_sb[:, c, :],
                        start=(c == 0), stop=(c == ct - 1))
                nc.any.tensor_copy(v_sb[:, mi, 0:OUT], ps[:])

        # ---- attention ----
        with tc.tile_pool(name="p_sb", bufs=2) as p_pool, \
             tc.tile_pool(name="s_ps", bufs=3, space="PSUM") as s_pool, \
             tc.tile_pool(name="y_ps", bufs=2, space="PSUM") as y_pool, \
             tc.tile_pool(name="t_ps", bufs=2, space="PSUM") as t_pool, \
             tc.tile_pool(name="fin", bufs=3) as fin_pool:
            for blk in range(nb):
                n0 = blk * nblk
                P_sb = p_pool.tile([128, mt, nblk], BF16, tag="p", name="P_sb")
                for mi in range(mt):
                    s_ps = s_pool.tile([128, nblk], F32, tag="s", name="s_t")
                    for o in range(ot):
                        nc.tensor.matmul(
                            s_ps[:], k_sb[:, o, mi * 128:(mi + 1) * 128],
                            q_sb[:, o, n0:n0 + nblk],
                            start=(o == 0), stop=(o == ot - 1))
                    nc.scalar.activation(P_sb[:, mi, :], s_ps[:], EXP,
                                         bias=zbias[:], scale=1.0)
                for ni in range(nt):
                    y_ps = y_pool.tile([128, OUT + 1], F32, tag="y", name="y_t")
                    for mi in range(mt):
                        nc.tensor.matmul(
                            y_ps[:], P_sb[:, mi, ni * 128:(ni + 1) * 128],
                            v_sb[:, mi, :],
                            start=(mi == 0), stop=(mi == mt - 1))
                    recip = fin_pool.tile([128, 1], F32, tag="recip",
                                          name="recip")
                    nc.vector.reciprocal(recip[:], y_ps[:, OUT:OUT + 1])
                    yn = fin_pool.tile([128, OUT], F32, tag="yn", name="yn")
                    nc.vector.tensor_scalar_mul(yn[:], y_ps[:, 0:OUT], recip[:])
                    for o in range(ot):
                        t_ps = t_pool.tile([128, 128], F32, tag="t", name="t_t")
                        nc.tensor.transpose(
                            t_ps[:], yn[:, o * 128:(o + 1) * 128], ident_sb[:])
                        nc.vector.tensor_copy(
                            y_sb[:, o, n0 + ni * 128:n0 + (ni + 1) * 128],
                            t_ps[:])
        for o in range(ot):
            nc.sync.dma_start(y_d.ap()[o], y_sb[:, o, :])
    nc.compile()
    return nc


def make_in_maps(x, xx, Wq, Wk, Wv, bc=2048, m=4096):
    """Host-side prep: slice/cast per-core inputs. Returns list of 8 dicts."""
    ct = C // 128
    wqT = (Wq.T / TEMP).astype(BFNP)   # fold 1/TEMP into Wq
    wkT = Wk.T.astype(BFNP)
    wvT = Wv.T.astype(BFNP)
    wq_t = np.ascontiguousarray(wqT.reshape(ct, 128, OUT))
    wk_t = np.ascontiguousarray(wkT.reshape(ct, 128, OUT))
    wv_t = np.ascontiguousarray(wvT.reshape(ct, 128, OUT))
    ident = np.eye(128, dtype=np.float32)
    halves = NCORES // B
    in_maps = []
    for core in range(NCORES):
        b, h = divmod(core, halves)
        xq = np.ascontiguousarray(
            x[b, :, h * bc:(h + 1) * bc].astype(BFNP).reshape(ct, 128, bc))
        xkv = np.ascontiguousarray(
            xx[b, :, :m].astype(BFNP).reshape(ct, 128, m))
        in_maps.append({"xq": xq, "xkv": xkv, "wqT": wq_t, "wkT": wk_t,
                        "wvT": wv_t, "ident": ident})
    return in_maps


def gather_output(results, bc=2048):
    """Reassemble per-core (ot, 128, bc) outputs into (B, OUT, NSEQ)."""
    y = np.empty((B, OUT, NSEQ), dtype=np.float32)
    halves = NCORES // B
    for core, res in enumerate(results):
        b, h = divmod(core, halves)
        y[b, :, h * bc:(h + 1) * bc] = res["y"].reshape(OUT, bc)
    return y


_NC_CACHE = {}


def kernel(x, xx, Wq, Wk, Wv):
    x = np.asarray(x)
    xx = np.asarray(xx)
    key = "full"
    if key not in _NC_CACHE:
        _NC_CACHE[key] = build()
    nc = _NC_CACHE[key]
    in_maps = make_in_maps(x, xx, np.asarray(Wq), np.asarray(Wk),
                           np.asarray(Wv))
    res = run_bass_kernel_spmd(nc, in_maps, core_ids=list(range(NCORES)))
    return gather_output(res.results)


# revision 9
# speedup vs baseline: 1.2904x; 1.2904x over previous
"""CrossAttention kernel for Trainium2 (8 NeuronCores, SPMD).

Reference math (B=4, C=256, N=4096, OUT=256, TEMP=sqrt(OUT)=16):
    q = Wq @ x          (B, OUT, N)
    k = Wk @ xx         (B, OUT, N)
    v = Wv @ xx         (B, OUT, N)
    attn = softmax(q^T k / TEMP, axis=-1)   (B, N, N)
    y = einsum('bnm,bom->bon', attn, v)     (B, OUT, N)

Sharding: 8 cores = (batch b, query-half h); each core computes its 2048
query rows against the full 4096 keys of its batch.

Per-core kernel (all matmuls bf16 in / fp32 PSUM accumulate):
    q   = (Wq/TEMP) @ x  -> (OUT, bc)      [o on partitions]
    k   = Wk @ xx        -> (OUT, m)       [o on partitions]
    vT  = xx^T @ Wv^T (+ones col) -> (m, OUT+1)  [m on partitions]
    S_T = k-tiles^T @ q  -> (m, bc) blocks [m on partitions]
    P_T = exp(S_T)  bf16 (logits are O(6) so no max-subtraction needed)
    yT  = P_T-tiles^T @ vT_aug -> (128n, OUT+1) psum; last col = denom
    y   = transpose(yT[:, :OUT] * 1/yT[:, OUT])  [o on partitions]

The S+exp stage of block b+1 is emitted before the PV stage of block b so
ScalarE's exp (427ns per [128,512] tile) hides under PE matmuls.
"""

import numpy as np
import ml_dtypes
from contextlib import ExitStack

import concourse.bass as bass
import concourse.tile as tile
from concourse import bacc, mybir
from concourse.bass_utils import run_bass_kernel_spmd

B, C, NSEQ, OUT = 4, 256, 4096, 256
TEMP = float(OUT) ** 0.5
NCORES = 8
BF16 = mybir.dt.bfloat16
F32 = mybir.dt.float32
BFNP = ml_dtypes.bfloat16

EXP = mybir.ActivationFunctionType.Exp


def build(bc=2048, m=4096, nblk=512, repeat_full=1):
    """Build the per-core SPMD Bass program.

    bc: query rows per core; m: key count; nblk: query block width
    (nblk*4B <= one PSUM bank). repeat_full: re-run the whole body R times
    (perf measurement only).
    """
    ct = C // 128     # contraction tiles for the 1x1-conv projections
    ot = OUT // 128   # output-channel tiles
    mt = m // 128     # key tiles
    nb = bc // nblk   # query blocks
    nt = nblk // 128  # 128-query tiles per block
    qch = bc // 512
    kch = m // 512

    nc = bacc.Bacc("TRN2", target_bir_lowering=False, debug=False,
                   num_devices=NCORES)
    xq_d = nc.dram_tensor("xq", [ct, 128, bc], BF16, kind="ExternalInput")
    xkv_d = nc.dram_tensor("xkv", [ct, 128, m], BF16, kind="ExternalInput")
    wq_d = nc.dram_tensor("wqT", [ct, 128, OUT], BF16, kind="ExternalInput")
    wk_d = nc.dram_tensor("wkT", [ct, 128, OUT], BF16, kind="ExternalInput")
    wv_d = nc.dram_tensor("wvT", [ct, 128, OUT], BF16, kind="ExternalInput")
    id_d = nc.dram_tensor("ident", [128, 128], F32, kind="ExternalInput")
    y_d = nc.dram_tensor("y", [ot, 128, bc], F32, kind="ExternalOutput")

    with tile.TileContext(nc) as tc, ExitStack() as ctx:
        const = ctx.enter_context(tc.tile_pool(name="const", bufs=1))

        xq_sb = const.tile([128, ct, bc], BF16, name="xq_sb")
        xkv_sb = const.tile([128, ct, m], BF16, name="xkv_sb")
        wq_sb = const.tile([128, ct, OUT], BF16, name="wq_sb")
        wk_sb = const.tile([128, ct, OUT], BF16, name="wk_sb")
        wv_sb = const.tile([128, ct, OUT], BF16, name="wv_sb")
        ident_sb = const.tile([128, 128], F32, name="ident_sb")
        zbias = const.tile([128, 1], F32, name="zbias")
        q_sb = const.tile([128, ot, bc], BF16, name="q_sb")
        k_sb = const.tile([128, ot, m], BF16, name="k_sb")
        v_sb = const.tile([128, mt, OUT + 1], BF16, name="v_sb")
        y_sb = const.tile([128, ot, bc], F32, name="y_sb")

        for _rf in range(repeat_full):
            # weights first (small; first matmuls need them), x chunked so
            # the projections can start before the full load finishes
            for i in range(ct):
                nc.sync.dma_start(wq_sb[:, i, :], wq_d.ap()[i])
                nc.sync.dma_start(wk_sb[:, i, :], wk_d.ap()[i])
                nc.sync.dma_start(wv_sb[:, i, :], wv_d.ap()[i])
            nc.sync.dma_start(ident_sb[:], id_d.ap())
            for i in range(ct):
                for chk in range(qch):
                    nc.sync.dma_start(
                        xq_sb[:, i, chk * 512:(chk + 1) * 512],
                        xq_d.ap()[i][:, chk * 512:(chk + 1) * 512])
                for chk in range(kch):
                    nc.sync.dma_start(
                        xkv_sb[:, i, chk * 512:(chk + 1) * 512],
                        xkv_d.ap()[i][:, chk * 512:(chk + 1) * 512])
            nc.vector.memset(zbias[:], 0.0)
            nc.vector.memset(v_sb[:, :, OUT:OUT + 1], 1.0)

            # ---- q / k / vT projections ----
            with tc.tile_pool(name="qk_ps", bufs=3, space="PSUM") as qk_pool, \
                 tc.tile_pool(name="v_ps", bufs=3, space="PSUM") as v_pool:
                for w_sb, x_sb, dst_sb, nch in ((wq_sb, xq_sb, q_sb, qch),
                                                (wk_sb, xkv_sb, k_sb, kch)):
                    for o in range(ot):
                        for chk in range(nch):
                            ps = qk_pool.tile([128, 512], F32, tag="qk",
                                              name="qk_t")
                            for c in range(ct):
                                nc.tensor.matmul(
                                    ps[:], w_sb[:, c, o * 128:(o + 1) * 128],
                                    x_sb[:, c, chk * 512:(chk + 1) * 512],
                                    start=(c == 0), stop=(c == ct - 1))
                            nc.vector.tensor_copy(
                                dst_sb[:, o, chk * 512:(chk + 1) * 512],
                                ps[:])
                for mi in range(mt):
                    ps = v_pool.tile([128, OUT], F32, tag="v", name="v_t")
                    for c in range(ct):
                        nc.tensor.matmul(
                            ps[:], xkv_sb[:, c, mi * 128:(mi + 1) * 128],
                            wv_sb[:, c, :],
                            start=(c == 0), stop=(c == ct - 1))
                    nc.vector.tensor_copy(v_sb[:, mi, 0:OUT], ps[:])

            # ---- attention ----
            with tc.tile_pool(name="p_sb", bufs=2) as p_pool, \
                 tc.tile_pool(name="s_ps", bufs=3, space="PSUM") as s_pool, \
                 tc.tile_pool(name="y_ps", bufs=3, space="PSUM") as y_pool, \
                 tc.tile_pool(name="t_ps", bufs=2, space="PSUM") as t_pool, \
                 tc.tile_pool(name="fin", bufs=3) as fin_pool:
                P_tiles = [None] * nb
                for blk in range(nb + 1):
                    if blk < nb:
                        # S_T = k^T q for block blk, exp -> P_T
                        n0 = blk * nblk
                        P_sb = p_pool.tile([128, mt, nblk], BF16, tag="p",
                                           name="P_sb")
                        P_tiles[blk] = P_sb
                        for mi in range(mt):
                            s_ps = s_pool.tile([128, nblk], F32, tag="s",
                                               name="s_t")
                            for o in range(ot):
                                nc.tensor.matmul(
                                    s_ps[:],
                                    k_sb[:, o, mi * 128:(mi + 1) * 128],
                                    q_sb[:, o, n0:n0 + nblk],
                                    start=(o == 0), stop=(o == ot - 1))
                            nc.scalar.activation(P_sb[:, mi, :], s_ps[:], EXP,
                                                 bias=zbias[:], scale=1.0)
                    if blk == 0:
                        continue
                    # PV stage for block blk-1
                    n0 = (blk - 1) * nblk
                    P_sb = P_tiles[blk - 1]
                    for ni in range(nt):
                        y_ps = y_pool.tile([128, OUT + 1], F32, tag="y",
                                           name="y_t")
                        for mi in range(mt):
                            nc.tensor.matmul(
                                y_ps[:],
                                P_sb[:, mi, ni * 128:(ni + 1) * 128],
                                v_sb[:, mi, :],
                                start=(mi == 0), stop=(mi == mt - 1))
                        recip = fin_pool.tile([128, 1], F32, tag="recip",
                                              name="recip")
                        nc.vector.reciprocal(recip[:], y_ps[:, OUT:OUT + 1])
                        yn = fin_pool.tile([128, OUT], F32, tag="yn",
                                           name="yn")
                        nc.vector.tensor_scalar_mul(yn[:], y_ps[:, 0:OUT],
                                                    recip[:])
                        for o in range(ot):
                            t_ps = t_pool.tile([128, 128], F32, tag="t",
                                               name="t_t")
                            nc.tensor.transpose(
                                t_ps[:], yn[:, o * 128:(o + 1) * 128],
                                ident_sb[:])
                            nc.vector.tensor_copy(
                                y_sb[:, o, n0 + ni * 128:n0 + (ni + 1) * 128],
                                t_ps[:])
        for o in range(ot):
            nc.sync.dma_start(y_d.ap()[o], y_sb[:, o, :])
    nc.compile()
    return nc


def make_in_maps(x, xx, Wq, Wk, Wv, bc=2048, m=4096):
    """Host-side prep: slice/cast per-core inputs. Returns list of 8 dicts."""
    ct = C // 128
    wqT = (Wq.T / TEMP).astype(BFNP)   # fold 1/TEMP into Wq
    wkT = Wk.T.astype(BFNP)
    wvT = Wv.T.astype(BFNP)
    wq_t = np.ascontiguousarray(wqT.reshape(ct, 128, OUT))
    wk_t = np.ascontiguousarray(wkT.reshape(ct, 128, OUT))
    wv_t = np.ascontiguousarray(wvT.reshape(ct, 128, OUT))
    ident = np.eye(128, dtype=np.float32)
    halves = NCORES // B
    in_maps = []
    for core in range(NCORES):
        b, h = divmod(core, halves)
        xq = np.ascontiguousarray(
            x[b, :, h * bc:(h + 1) * bc].astype(BFNP).reshape(ct, 128, bc))
        xkv = np.ascontiguousarray(
            xx[b, :, :m].astype(BFNP).reshape(ct, 128, m))
        in_maps.append({"xq": xq, "xkv": xkv, "wqT": wq_t, "wkT": wk_t,
                        "wvT": wv_t, "ident": ident})
    return in_maps


def gather_output(results, bc=2048):
    """Reassemble per-core (ot, 128, bc) outputs into (B, OUT, NSEQ)."""
    y = np.empty((B, OUT, NSEQ), dtype=np.float32)
    halves = NCORES // B
    for core, res in enumerate(results):
        b, h = divmod(core, halves)
        y[b, :, h * bc:(h + 1) * bc] = res["y"].reshape(OUT, bc)
    return y


_NC_CACHE = {}


def kernel(x, xx, Wq, Wk, Wv):
    x = np.asarray(x)
    xx = np.asarray(xx)
    key = "full"
    if key not in _NC_CACHE:
        _NC_CACHE[key] = build()
    nc = _NC_CACHE[key]
    in_maps = make_in_maps(x, xx, np.asarray(Wq), np.asarray(Wk),
                           np.asarray(Wv))
    res = run_bass_kernel_spmd(nc, in_maps, core_ids=list(range(NCORES)))
    return gather_output(res.results)


# revision 20
# speedup vs baseline: 57.8041x; 44.7943x over previous
"""CrossAttention kernel for Trainium2 (8 NeuronCores, SPMD).

Reference math (B=4, C=256, N=4096, OUT=256, TEMP=sqrt(OUT)=16):
    q = Wq @ x          (B, OUT, N)
    k = Wk @ xx         (B, OUT, N)
    v = Wv @ xx         (B, OUT, N)
    attn = softmax(q^T k / TEMP, axis=-1)   (B, N, N)
    y = einsum('bnm,bom->bon', attn, v)     (B, OUT, N)

Sharding: 8 cores = (batch b, query-half h); each core computes its 2048
query rows against the full 4096 keys of its batch.

Per-core kernel (all matmuls bf16 in / fp32 PSUM accumulate):
    q   = (Wq/TEMP) @ x  -> (OUT, bc)      [o on partitions]
    k   = Wk @ xx        -> (OUT, m)       [o on partitions]
    vT  = xx^T @ Wv^T (+ones col) -> (m, OUT+1)  [m on partitions]
    S_T = k-tiles^T @ q  -> (m, bc) blocks [m on partitions]
    P_T = exp(S_T)  bf16 (logits are O(6) so no max-subtraction needed)
    yT  = P_T-tiles^T @ vT_aug -> (128n, OUT+1) psum; last col = denom
    y   = transpose(yT[:, :OUT] * 1/yT[:, OUT])  [o on partitions]

The S+exp stage of block b+1 is emitted before the PV stage of block b so
ScalarE's exp (427ns per [128,512] tile) hides under PE matmuls.
"""

import numpy as np
import ml_dtypes
from contextlib import ExitStack

import concourse.bass as bass
import concourse.tile as tile
from concourse import bacc, mybir
from concourse.bass_utils import run_bass_kernel_spmd

B, C, NSEQ, OUT = 4, 256, 4096, 256
TEMP = float(OUT) ** 0.5
NCORES = 8
BF16 = mybir.dt.bfloat16
F32 = mybir.dt.float32
BFNP = ml_dtypes.bfloat16

EXP = mybir.ActivationFunctionType.Exp


def build(bc=2048, m=4096, nblk=512, repeat_full=1):
    """Build the per-core SPMD Bass program.

    bc: query rows per core; m: key count; nblk: query block width
    (nblk*4B <= one PSUM bank). repeat_full: re-run the whole body R times
    (perf measurement only).
    """
    ct = C // 128     # contraction tiles for the 1x1-conv projections
    ot = OUT // 128   # output-channel tiles
    mt = m // 128     # key tiles
    nb = bc // nblk   # query blocks
    nt = nblk // 128  # 128-query tiles per block
    qch = bc // 512
    kch = m // 512

    nc = bacc.Bacc("TRN2", target_bir_lowering=False, debug=False,
                   num_devices=NCORES)
    xq_d = nc.dram_tensor("xq", [ct, 128, bc], BF16, kind="ExternalInput")
    xkv_d = nc.dram_tensor("xkv", [ct, 128, m], BF16, kind="ExternalInput")
    wq_d = nc.dram_tensor("wqT", [ct, 128, OUT], BF16, kind="ExternalInput")
    wk_d = nc.dram_tensor("wkT", [ct, 128, OUT], BF16, kind="ExternalInput")
    wv_d = nc.dram_tensor("wvT", [ct, 128, OUT], BF16, kind="ExternalInput")
    id_d = nc.dram_tensor("ident", [128, 128], F32, kind="ExternalInput")
    y_d = nc.dram_tensor("y", [ot, 128, bc], F32, kind="ExternalOutput")

    with tile.TileContext(nc) as tc, ExitStack() as ctx:
        const = ctx.enter_context(tc.tile_pool(name="const", bufs=1))

        xq_sb = const.tile([128, ct, bc], BF16, name="xq_sb")
        xkv_sb = const.tile([128, ct, m], BF16, name="xkv_sb")
        wq_sb = const.tile([128, ct, OUT], BF16, name="wq_sb")
        wk_sb = const.tile([128, ct, OUT], BF16, name="wk_sb")
        wv_sb = const.tile([128, ct, OUT], BF16, name="wv_sb")
        ident_sb = const.tile([128, 128], F32, name="ident_sb")
        zbias = const.tile([128, 1], F32, name="zbias")
        q_sb = const.tile([128, ot, bc], BF16, name="q_sb")
        k_sb = const.tile([128, ot, m], BF16, name="k_sb")
        v_sb = const.tile([128, mt, OUT + 1], BF16, name="v_sb")
        y_sb = const.tile([128, ot, bc], F32, name="y_sb")

        for _rf in range(repeat_full):
            # weights first (small; first matmuls need them), x chunked and
            # c-tiles interleaved so each projection matmul unblocks as soon
            # as its own chunk pair has landed
            for i in range(ct):
                nc.sync.dma_start(wq_sb[:, i, :], wq_d.ap()[i])
            for chk in range(qch):
                for i in range(ct):
                    nc.sync.dma_start(
                        xq_sb[:, i, chk * 512:(chk + 1) * 512],
                        xq_d.ap()[i][:, chk * 512:(chk + 1) * 512])
            for i in range(ct):
                nc.sync.dma_start(wk_sb[:, i, :], wk_d.ap()[i])
            for chk in range(kch):
                for i in range(ct):
                    nc.sync.dma_start(
                        xkv_sb[:, i, chk * 512:(chk + 1) * 512],
                        xkv_d.ap()[i][:, chk * 512:(chk + 1) * 512])
                if chk == 0:
                    for i in range(ct):
                        nc.sync.dma_start(wv_sb[:, i, :], wv_d.ap()[i])
            nc.sync.dma_start(ident_sb[:], id_d.ap())
            nc.vector.memset(zbias[:], 0.0)
            nc.vector.memset(v_sb[:, :, OUT:OUT + 1], 1.0)

            # ---- q / k / vT projections ----
            with tc.tile_pool(name="qk_ps", bufs=3, space="PSUM") as qk_pool, \
                 tc.tile_pool(name="v_ps", bufs=3, space="PSUM") as v_pool:
                for o in range(ot):
                    for chk in range(qch):
                        ps = qk_pool.tile([128, 512], F32, tag="qk",
                                          name="qk_t")
                        for c in range(ct):
                            nc.tensor.matmul(
                                ps[:], wq_sb[:, c, o * 128:(o + 1) * 128],
                                xq_sb[:, c, chk * 512:(chk + 1) * 512],
                                start=(c == 0), stop=(c == ct - 1))
                        nc.vector.tensor_copy(
                            q_sb[:, o, chk * 512:(chk + 1) * 512], ps[:])
                # k and v interleaved by xkv chunk so PE work follows the
                # DMA arrival order
                for chk in range(kch):
                    for o in range(ot):
                        ps = qk_pool.tile([128, 512], F32, tag="qk",
                                          name="qk_t")
                        for c in range(ct):
                            nc.tensor.matmul(
                                ps[:], wk_sb[:, c, o * 128:(o + 1) * 128],
                                xkv_sb[:, c, chk * 512:(chk + 1) * 512],
                                start=(c == 0), stop=(c == ct - 1))
                        nc.vector.tensor_copy(
                            k_sb[:, o, chk * 512:(chk + 1) * 512], ps[:])
                    for mi in range(4 * chk, 4 * (chk + 1)):
                        ps = v_pool.tile([128, OUT], F32, tag="v", name="v_t")
                        for c in range(ct):
                            nc.tensor.matmul(
                                ps[:], xkv_sb[:, c, mi * 128:(mi + 1) * 128],
                                wv_sb[:, c, :],
                                start=(c == 0), stop=(c == ct - 1))
                        # ACT (idle here; all Copy-activates land before the
                        # first Exp, so only one table switch)
                        nc.scalar.copy(v_sb[:, mi, 0:OUT], ps[:])

            # ---- attention ----
            with tc.tile_pool(name="p_sb", bufs=2) as p_pool, \
                 tc.tile_pool(name="s_ps", bufs=2, space="PSUM") as s_pool, \
                 tc.tile_pool(name="y_ps", bufs=2, space="PSUM") as y_pool, \
                 tc.tile_pool(name="t_ps", bufs=2, space="PSUM") as t_pool, \
                 tc.tile_pool(name="fin", bufs=3) as fin_pool:
                P_tiles = [None] * nb
                pending = []   # deferred transposes: (yn_tile, global ni)

                def flush_transposes():
                    # batched so the PE pays few matmul<->transpose mode
                    # transitions; per-block output DMA afterwards
                    for yn_t, gni in pending:
                        for o in range(ot):
                            t_ps = t_pool.tile([128, 128], F32, tag="t",
                                               name="t_t")
                            nc.tensor.transpose(
                                t_ps[:], yn_t[:, o * 128:(o + 1) * 128],
                                ident_sb[:])
                            nc.vector.tensor_copy(
                                y_sb[:, o, gni * 128:(gni + 1) * 128],
                                t_ps[:])
                    if pending:
                        n0 = (pending[0][1] // nt) * nblk
                        for o in range(ot):
                            nc.sync.dma_start(
                                y_d.ap()[o][:, n0:n0 + nblk],
                                y_sb[:, o, n0:n0 + nblk])
                    pending.clear()

                for blk in range(nb + 1):
                    if blk < nb:
                        # S_T = k^T q for block blk, exp -> P_T
                        # m-tiles paired: one [128, 2, nblk] psum tile
                        # (2 banks), one exp per pair
                        n0 = blk * nblk
                        P_sb = p_pool.tile([128, mt, nblk], BF16, tag="p",
                                           name="P_sb")
                        P_tiles[blk] = P_sb
                        for mj in range(mt // 2):
                            s_ps = s_pool.tile([128, 2, nblk], F32, tag="s",
                                               name="s_t")
                            for half in range(2):
                                mi = 2 * mj + half
                                for o in range(ot):
                                    nc.tensor.matmul(
                                        s_ps[:, half, :],
                                        k_sb[:, o, mi * 128:(mi + 1) * 128],
                                        q_sb[:, o, n0:n0 + nblk],
                                        start=(o == 0), stop=(o == ot - 1))
                            nc.scalar.activation(
                                P_sb[:, 2 * mj:2 * mj + 2, :], s_ps[:], EXP,
                                bias=zbias[:], scale=1.0)
                    if blk == 0:
                        continue
                    # PV stage for block blk-1; transposes of blk-2 are
                    # flushed after these PV matmuls so the PE never waits
                    # on the DVE normalize chain
                    prev_pending, pending = pending, []
                    P_sb = P_tiles[blk - 1]
                    for ni in range(nt):
                        y_ps = y_pool.tile([128, OUT + 1], F32, tag="y",
                                           name="y_t")
                        for mi in range(mt):
                            nc.tensor.matmul(
                                y_ps[:],
                                P_sb[:, mi, ni * 128:(ni + 1) * 128],
                                v_sb[:, mi, :],
                                start=(mi == 0), stop=(mi == mt - 1))
                        recip = fin_pool.tile([128, 1], F32, tag="recip",
                                              name="recip")
                        nc.vector.reciprocal(recip[:], y_ps[:, OUT:OUT + 1])
                        yn = fin_pool.tile([128, OUT], F32, tag="yn",
                                           name="yn", bufs=8)
                        nc.vector.tensor_scalar_mul(yn[:], y_ps[:, 0:OUT],
                                                    recip[:])
                        pending.append((yn, (blk - 1) * nt + ni))
                    prev_pending, pending = pending, prev_pending
                    flush_transposes()
                    pending = prev_pending
                flush_transposes()
    nc.compile()
    return nc


def make_in_maps(x, xx, Wq, Wk, Wv, bc=2048, m=4096):
    """Host-side prep: slice/cast per-core inputs. Returns list of 8 dicts."""
    ct = C // 128
    wqT = (Wq.T / TEMP).astype(BFNP)   # fold 1/TEMP into Wq
    wkT = Wk.T.astype(BFNP)
    wvT = Wv.T.astype(BFNP)
    wq_t = np.ascontiguousarray(wqT.reshape(ct, 128, OUT))
    wk_t = np.ascontiguousarray(wkT.reshape(ct, 128, OUT))
    wv_t = np.ascontiguousarray(wvT.reshape(ct, 128, OUT))
    ident = np.eye(128, dtype=np.float32)
    halves = NCORES // B
    in_maps = []
    for core in range(NCORES):
        b, h = divmod(core, halves)
        xq = np.ascontiguousarray(
            x[b, :, h * bc:(h + 1) * bc].astype(BFNP).reshape(ct, 128, bc))
        xkv = np.ascontiguousarray(
            xx[b, :, :m].astype(BFNP).reshape(ct, 128, m))
        in_maps.append({"xq": xq, "xkv": xkv, "wqT": wq_t, "wkT": wk_t,
                        "wvT": wv_t, "ident": ident})
    return in_maps


def gather_output(results, bc=2048):
    """Reassemble per-core (ot, 128, bc) outputs into (B, OUT, NSEQ)."""
    y = np.empty((B, OUT, NSEQ), dtype=np.float32)
    halves = NCORES // B
    for core, res in enumerate(results):
        b, h = divmod(core, halves)
        y[b, :, h * bc:(h + 1) * bc] = res["y"].reshape(OUT, bc)
    return y


_NC_CACHE = {}


def kernel(x, xx, Wq, Wk, Wv):
    x = np.asarray(x)
    xx = np.asarray(xx)
    key = "full"
    if key not in _NC_CACHE:
        _NC_CACHE[key] = build()
    nc = _NC_CACHE[key]
    in_maps = make_in_maps(x, xx, np.asarray(Wq), np.asarray(Wk),
                           np.asarray(Wv))
    res = run_bass_kernel_spmd(nc, in_maps, core_ids=list(range(NCORES)))
    return gather_output(res.results)


# revision 21
# speedup vs baseline: 62.7971x; 1.0864x over previous
"""CrossAttention kernel for Trainium2 (8 NeuronCores, SPMD).

Reference math (B=4, C=256, N=4096, OUT=256, TEMP=sqrt(OUT)=16):
    q = Wq @ x          (B, OUT, N)
    k = Wk @ xx         (B, OUT, N)
    v = Wv @ xx         (B, OUT, N)
    attn = softmax(q^T k / TEMP, axis=-1)   (B, N, N)
    y = einsum('bnm,bom->bon', attn, v)     (B, OUT, N)

Sharding: 8 cores = (batch b, query-half h); each core computes its 2048
query rows against the full 4096 keys of its batch.

Per-core kernel (all matmuls bf16 in / fp32 PSUM accumulate):
    q   = (Wq/TEMP) @ x  -> (OUT, bc)      [o on partitions]
    k   = Wk @ xx        -> (OUT, m)       [o on partitions]
    vT  = xx^T @ Wv^T (+ones col) -> (m, OUT+1)  [m on partitions]
    S_T = k-tiles^T @ q  -> (m, bc) blocks [m on partitions]
    P_T = exp(S_T)  bf16 (logits are O(6) so no max-subtraction needed)
    yT  = P_T-tiles^T @ vT_aug -> (128n, OUT+1) psum; last col = denom
    y   = transpose(yT[:, :OUT] * 1/yT[:, OUT])  [o on partitions]

The S+exp stage of block b+1 is emitted before the PV stage of block b so
ScalarE's exp (427ns per [128,512] tile) hides under PE matmuls.
"""

import numpy as np
import ml_dtypes
from contextlib import ExitStack

import concourse.bass as bass
import concourse.tile as tile
from concourse import bacc, mybir
from concourse.bass_utils import run_bass_kernel_spmd

B, C, NSEQ, OUT = 4, 256, 4096, 256
TEMP = float(OUT) ** 0.5
NCORES = 8
BF16 = mybir.dt.bfloat16
F32 = mybir.dt.float32
BFNP = ml_dtypes.bfloat16

EXP = mybir.ActivationFunctionType.Exp


def build(bc=2048, m=4096, nblk=512, repeat_full=1):
    """Build the per-core SPMD Bass program.

    bc: query rows per core; m: key count; nblk: query block width
    (nblk*4B <= one PSUM bank). repeat_full: re-run the whole body R times
    (perf measurement only).
    """
    ct = C // 128     # contraction tiles for the 1x1-conv projections
    ot = OUT // 128   # output-channel tiles
    mt = m // 128     # key tiles
    nb = bc // nblk   # query blocks
    nt = nblk // 128  # 128-query tiles per block
    qch = bc // 512
    kch = m // 512

    nc = bacc.Bacc("TRN2", target_bir_lowering=False, debug=False,
                   num_devices=NCORES)
    xq_d = nc.dram_tensor("xq", [ct, 128, bc], BF16, kind="ExternalInput")
    xkv_d = nc.dram_tensor("xkv", [ct, 128, m], BF16, kind="ExternalInput")
    wq_d = nc.dram_tensor("wqT", [ct, 128, OUT], BF16, kind="ExternalInput")
    wk_d = nc.dram_tensor("wkT", [ct, 128, OUT], BF16, kind="ExternalInput")
    wv_d = nc.dram_tensor("wvT", [ct, 128, OUT], BF16, kind="ExternalInput")
    id_d = nc.dram_tensor("ident", [128, 128], F32, kind="ExternalInput")
    y_d = nc.dram_tensor("y", [ot, 128, bc], F32, kind="ExternalOutput")

    with tile.TileContext(nc) as tc, ExitStack() as ctx:
        const = ctx.enter_context(tc.tile_pool(name="const", bufs=1))

        xq_sb = const.tile([128, ct, bc], BF16, name="xq_sb")
        xkv_sb = const.tile([128, ct, m], BF16, name="xkv_sb")
        wq_sb = const.tile([128, ct, OUT], BF16, name="wq_sb")
        wk_sb = const.tile([128, ct, OUT], BF16, name="wk_sb")
        wv_sb = const.tile([128, ct, OUT], BF16, name="wv_sb")
        ident_sb = const.tile([128, 128], F32, name="ident_sb")
        zbias = const.tile([128, 1], F32, name="zbias")
        q_sb = const.tile([128, ot, bc], BF16, name="q_sb")
        k_sb = const.tile([128, ot, m], BF16, name="k_sb")
        v_sb = const.tile([128, mt, OUT + 1], BF16, name="v_sb")
        y_sb = const.tile([128, ot, bc], F32, name="y_sb")

        for _rf in range(repeat_full):
            # weights first (small; first matmuls need them), x chunked and
            # c-tiles interleaved so each projection matmul unblocks as soon
            # as its own chunk pair has landed
            for i in range(ct):
                nc.sync.dma_start(wq_sb[:, i, :], wq_d.ap()[i])
            for chk in range(qch):
                for i in range(ct):
                    nc.sync.dma_start(
                        xq_sb[:, i, chk * 512:(chk + 1) * 512],
                        xq_d.ap()[i][:, chk * 512:(chk + 1) * 512])
            for i in range(ct):
                nc.sync.dma_start(wk_sb[:, i, :], wk_d.ap()[i])
            for chk in range(kch):
                for i in range(ct):
                    nc.sync.dma_start(
                        xkv_sb[:, i, chk * 512:(chk + 1) * 512],
                        xkv_d.ap()[i][:, chk * 512:(chk + 1) * 512])
                if chk == 0:
                    for i in range(ct):
                        nc.sync.dma_start(wv_sb[:, i, :], wv_d.ap()[i])
            nc.sync.dma_start(ident_sb[:], id_d.ap())
            nc.vector.memset(zbias[:], 0.0)
            nc.vector.memset(v_sb[:, :, OUT:OUT + 1], 1.0)

            # ---- q / k / vT projections ----
            with tc.tile_pool(name="qk_ps", bufs=3, space="PSUM") as qk_pool, \
                 tc.tile_pool(name="v_ps", bufs=3, space="PSUM") as v_pool:
                for o in range(ot):
                    for chk in range(qch):
                        ps = qk_pool.tile([128, 512], F32, tag="qk",
                                          name="qk_t")
                        for c in range(ct):
                            nc.tensor.matmul(
                                ps[:], wq_sb[:, c, o * 128:(o + 1) * 128],
                                xq_sb[:, c, chk * 512:(chk + 1) * 512],
                                start=(c == 0), stop=(c == ct - 1))
                        nc.vector.tensor_copy(
                            q_sb[:, o, chk * 512:(chk + 1) * 512], ps[:])
                # k and v interleaved by xkv chunk so PE work follows the
                # DMA arrival order
                for chk in range(kch):
                    for o in range(ot):
                        ps = qk_pool.tile([128, 512], F32, tag="qk",
                                          name="qk_t")
                        for c in range(ct):
                            nc.tensor.matmul(
                                ps[:], wk_sb[:, c, o * 128:(o + 1) * 128],
                                xkv_sb[:, c, chk * 512:(chk + 1) * 512],
                                start=(c == 0), stop=(c == ct - 1))
                        nc.vector.tensor_copy(
                            k_sb[:, o, chk * 512:(chk + 1) * 512], ps[:])
                    for mi in range(4 * chk, 4 * (chk + 1)):
                        ps = v_pool.tile([128, OUT], F32, tag="v", name="v_t")
                        for c in range(ct):
                            nc.tensor.matmul(
                                ps[:], xkv_sb[:, c, mi * 128:(mi + 1) * 128],
                                wv_sb[:, c, :],
                                start=(c == 0), stop=(c == ct - 1))
                        # ACT (idle here; all Copy-activates land before the
                        # first Exp, so only one table switch)
                        nc.scalar.copy(v_sb[:, mi, 0:OUT], ps[:])

            # ---- attention ----
            with tc.tile_pool(name="p_sb", bufs=2) as p_pool, \
                 tc.tile_pool(name="s_ps", bufs=2, space="PSUM") as s_pool, \
                 tc.tile_pool(name="y_ps", bufs=2, space="PSUM") as y_pool, \
                 tc.tile_pool(name="t_ps", bufs=2, space="PSUM") as t_pool, \
                 tc.tile_pool(name="fin", bufs=3) as fin_pool:
                P_tiles = [None] * nb
                pending = []   # deferred transposes: (yn_tile, global ni)

                def flush_transposes():
                    # batched so the PE pays few matmul<->transpose mode
                    # transitions; per-block output DMA afterwards
                    for yn_t, gni in pending:
                        for o in range(ot):
                            t_ps = t_pool.tile([128, 128], F32, tag="t",
                                               name="t_t")
                            nc.tensor.transpose(
                                t_ps[:], yn_t[:, o * 128:(o + 1) * 128],
                                ident_sb[:])
                            nc.vector.tensor_copy(
                                y_sb[:, o, gni * 128:(gni + 1) * 128],
                                t_ps[:])
                    if pending:
                        n0 = (pending[0][1] // nt) * nblk
                        for o in range(ot):
                            nc.sync.dma_start(
                                y_d.ap()[o][:, n0:n0 + nblk],
                                y_sb[:, o, n0:n0 + nblk])
                    pending.clear()

                for blk in range(nb + 1):
                    if blk < nb:
                        # S_T = k^T q for block blk, exp -> P_T
                        # m-tiles paired: one [128, 2, nblk] psum tile
                        # (2 banks), one exp per pair
                        n0 = blk * nblk
                        P_sb = p_pool.tile([128, mt, nblk], BF16, tag="p",
                                           name="P_sb")
                        P_tiles[blk] = P_sb
                        for mj in range(mt // 2):
                            s_ps = s_pool.tile([128, 2, nblk], F32, tag="s",
                                               name="s_t")
                            for half in range(2):
                                mi = 2 * mj + half
                                for o in range(ot):
                                    nc.tensor.matmul(
                                        s_ps[:, half, :],
                                        k_sb[:, o, mi * 128:(mi + 1) * 128],
                                        q_sb[:, o, n0:n0 + nblk],
                                        start=(o == 0), stop=(o == ot - 1))
                            nc.scalar.activation(
                                P_sb[:, 2 * mj:2 * mj + 2, :], s_ps[:], EXP,
                                bias=zbias[:], scale=1.0)
                    if blk == 0:
                        continue
                    # PV stage for block blk-1; transposes of blk-2 are
                    # flushed after these PV matmuls so the PE never waits
                    # on the DVE normalize chain
                    prev_pending, pending = pending, []
                    P_sb = P_tiles[blk - 1]
                    for ni in range(nt):
                        y_ps = y_pool.tile([128, OUT + 1], F32, tag="y",
                                           name="y_t")
                        for mi in range(mt):
                            nc.tensor.matmul(
                                y_ps[:],
                                P_sb[:, mi, ni * 128:(ni + 1) * 128],
                                v_sb[:, mi, :],
                                start=(mi == 0), stop=(mi == mt - 1))
                        recip = fin_pool.tile([128, 1], F32, tag="recip",
                                              name="recip")
                        nc.vector.reciprocal(recip[:], y_ps[:, OUT:OUT + 1])
                        yn = fin_pool.tile([128, OUT], F32, tag="yn",
                                           name="yn", bufs=8)
                        nc.vector.tensor_scalar_mul(yn[:], y_ps[:, 0:OUT],
                                                    recip[:])
                        pending.append((yn, (blk - 1) * nt + ni))
                    prev_pending, pending = pending, prev_pending
                    flush_transposes()
                    pending = prev_pending
                flush_transposes()
    nc.compile()
    return nc


def make_in_maps(x, xx, Wq, Wk, Wv, bc=2048, m=4096):
    """Host-side prep: slice/cast per-core inputs. Returns list of 8 dicts."""
    ct = C // 128
    wqT = (Wq.T / TEMP).astype(BFNP)   # fold 1/TEMP into Wq
    wkT = Wk.T.astype(BFNP)
    wvT = Wv.T.astype(BFNP)
    wq_t = np.ascontiguousarray(wqT.reshape(ct, 128, OUT))
    wk_t = np.ascontiguousarray(wkT.reshape(ct, 128, OUT))
    wv_t = np.ascontiguousarray(wvT.reshape(ct, 128, OUT))
    ident = np.eye(128, dtype=np.float32)
    halves = NCORES // B
    in_maps = []
    for core in range(NCORES):
        b, h = divmod(core, halves)
        xq = np.ascontiguousarray(
            x[b, :, h * bc:(h + 1) * bc].astype(BFNP).reshape(ct, 128, bc))
        xkv = np.ascontiguousarray(
            xx[b, :, :m].astype(BFNP).reshape(ct, 128, m))
        in_maps.append({"xq": xq, "xkv": xkv, "wqT": wq_t, "wkT": wk_t,
                        "wvT": wv_t, "ident": ident})
    return in_maps


def gather_output(results, bc=2048):
    """Reassemble per-core (ot, 128, bc) outputs into (B, OUT, NSEQ)."""
    y = np.empty((B, OUT, NSEQ), dtype=np.float32)
    halves = NCORES // B
    for core, res in enumerate(results):
        b, h = divmod(core, halves)
        y[b, :, h * bc:(h + 1) * bc] = res["y"].reshape(OUT, bc)
    return y


_NC_CACHE = {}


def kernel(x, xx, Wq, Wk, Wv):
    x = np.asarray(x)
    xx = np.asarray(xx)
    key = "full"
    if key not in _NC_CACHE:
        _NC_CACHE[key] = build()
    nc = _NC_CACHE[key]
    in_maps = make_in_maps(x, xx, np.asarray(Wq), np.asarray(Wk),
                           np.asarray(Wv))
    try:
        res = run_bass_kernel_spmd(nc, in_maps, core_ids=list(range(NCORES)))
    except Exception:
        # transient device state (e.g. a previous process left a core
        # unrecoverable) usually clears on retry
        res = run_bass_kernel_spmd(nc, in_maps, core_ids=list(range(NCORES)))
    return gather_output(res.results)
